# revision 21
# baseline (speedup 1.0000x reference)
"""MetaSR super-resolution Trainium2 kernel (bf16, dual-queue, hybrid taps).

Structure exploited: out_h=out_w=256 with H=W=64 LR grid means the scale
factor is exactly 4, so the nearest-neighbor gather index is iy=oy//4,
ix=ox//4 and the per-query MLP input collapses to 16 distinct subpixel
phases [dy/4, dx/4, 0.25].  The whole model becomes:

  1. h    = relu(mlp_in @ w1 + b1)              [16, 256]
  2. predw = h @ w2 + b2                        [16, 576, 3]
  3. rgb[o, 4*iy+dy, 4*ix+dx] =
       sum_{c,ki,kj} feat[c, iy+ki-1, ix+kj-1] * predw[(dy,dx), c*9+ki*3+kj, o]
     i.e. a 3x3 conv with 64 in / 48 out channels + pixel shuffle.

Sharding: data-parallel over LR rows (8 rows per core, 10-row halo band),
weights replicated; steps 1+2 are recomputed on every core (tiny).

Conv decomposition: one [128, 663] band tile holds the zero-padded band
twice -- partitions 0-63 at free-offset 1, partitions 64-127 at offset 0.
That single tile supports K=128 tap pairs with shift delta 1 ([t0;t1],
[t4;t5], [t6;t7]) and the remaining taps t2/t3/t8 singly (K=64 from the
offset-1 half), for 6 conv matmuls off one 169KB band transfer.

The b2 bias rides each w2 chunk blob as a prebroadcast [K, 48] tile; one
DVE tensor_add per chunk applies bias + bf16 cast, and each chunk's conv
is issued one chunk BEHIND its assembly (software pipelining) so that DVE
op is never on the PE's critical path.

All large operands are bf16 (host-side cast); verified end-to-end rel err
~4e-3 vs the f32 reference (budget 2e-2).

DMA: measured ~215-230GB/s per queue with ~0.3us per-transfer bubbles, and
near-360GB/s aggregate when BOTH hardware queues stream concurrently.  So:
Sync carries [w1+mlp+b1 | w2 chunk0] then [chunk1|chunk2] then the output;
Scalar carries [band] then [chunks 3-5], but its first trigger is GATED on
the arrival of the first Sync blob (a 1-element scalar-engine copy) so the
critical MLP weights always win the engine race.  Dummy "filler" matmuls
with explicit deps keep the PE busy across the remaining DMA waits.
"""

import os

import numpy as np

try:
    import concourse.bass as bass
except ImportError:  # fall back to the repo checkout
    import sys
    sys.path.insert(0, "/opt/trn_rl_repo")
    import concourse.bass as bass
import concourse.mybir as mybir
import concourse.tile as tile
from concourse import bacc
from concourse.bass_utils import run_bass_kernel_spmd

F32 = mybir.dt.float32
BF16 = mybir.dt.bfloat16
N_CORES = 8
ROWS_PER_CORE = 8          # LR rows per core
BAND_ROWS = ROWS_PER_CORE + 2
NPOS = ROWS_PER_CORE * 64  # 512 LR positions per core

# Taps t = ki*3+kj have band shift ki*66+kj:
#   t:      0   1   2   3    4    5    6    7    8
#   shift:  0   1   2   66   67   68   132  133  134
# chunks: ([taps], part_lo, rhs_offset, K)
CHUNK_SPECS = [
    ([0, 1], 0, 1, 128),
    ([4, 5], 0, 68, 128),
    ([6, 7], 0, 133, 128),
    ([2], 0, 3, 64),
    ([3], 0, 67, 64),
    ([8], 0, 135, 64),
]
N_CHUNKS = len(CHUNK_SPECS)

# blob layout (bf16 cols); per-chunk w2 block = 6*[128,K] + [K,48] b2 tile
OFF_W1 = 4            # b1 takes 2 f32 = 4 bf16 cols
OFF_MLP = 4 + 256
COLS_W1X = 4 + 256 + 16           # = 276
CHUNK_COLS = [6 * spec[3] + 48 for spec in CHUNK_SPECS]   # 816/816/816/432/432/432
COLS_A = COLS_W1X + CHUNK_COLS[0]          # w1x | chunk0
COLS_B = CHUNK_COLS[1] + CHUNK_COLS[2]     # chunk1 | chunk2
COLS_E = CHUNK_COLS[3] + CHUNK_COLS[4] + CHUNK_COLS[5]
COLS_BAND = 663


def _env(name, default):
    return int(os.environ.get(name, str(default)))

WARM_BIG = _env("METASR_WARM_BIG", 2)
WARM_SMALL = _env("METASR_WARM_SMALL", 2)
# filler counts: after mlp1, then after each chunk's W assembly
FILLS = [int(x) for x in os.environ.get("METASR_FILLS", "3,2,1,1,1,1,0").split(",")]

_CACHE = {}


def _build_program(cfg):
    """Build + compile the single-core Bass program (same for all cores)."""
    warm_big, warm_small, fills = cfg[0], cfg[1], list(cfg[2])
    nc = bacc.Bacc("TRN2", target_bir_lowering=False, debug=False)

    blob_a_d = nc.dram_tensor("blob_a", [128, COLS_A], BF16, kind="ExternalInput")
    blob_b_d = nc.dram_tensor("blob_b", [128, COLS_B], BF16, kind="ExternalInput")
    blob_band_d = nc.dram_tensor("blob_band", [128, COLS_BAND], BF16, kind="ExternalInput")
    blob_e_d = nc.dram_tensor("blob_e", [128, COLS_E], BF16, kind="ExternalInput")
    out48 = nc.dram_tensor("out48", [48, NPOS], BF16, kind="ExternalOutput")

    with tile.TileContext(nc) as tc:
        with (
            tc.tile_pool(name="blobs", bufs=1) as blobs,
            tc.tile_pool(name="work", bufs=1) as work,
            tc.tile_pool(name="wpool", bufs=6) as wpool,
            tc.tile_pool(name="opool", bufs=1) as opool,
            tc.tile_pool(name="ps_small", bufs=2, space="PSUM") as ps_small,
            tc.tile_pool(name="ps_w", bufs=4, space="PSUM") as ps_w,
            tc.tile_pool(name="ps_warm", bufs=1, space="PSUM") as ps_warm,
            tc.tile_pool(name="ps_rgb", bufs=1, space="PSUM") as ps_rgb,
        ):
            # Sync queue: critical stream.
            blob_a = blobs.tile([128, COLS_A], BF16, tag="blob_a")
            nc.sync.dma_start(blob_a[:, :], blob_a_d[:, :])
            blob_b = blobs.tile([128, COLS_B], BF16, tag="blob_b")
            nc.sync.dma_start(blob_b[:, :], blob_b_d[:, :])
            # Scalar queue: gated on blob_a's arrival so Sync wins the race.
            gate = work.tile([1, 1], BF16, tag="gate")
            nc.scalar.copy(gate[:, :], blob_a[0:1, 0:1])
            blob_band = blobs.tile([128, COLS_BAND], BF16, tag="band")
            nc.scalar.dma_start(blob_band[:, :], blob_band_d[:, :])
            blob_e = blobs.tile([128, COLS_E], BF16, tag="blob_e")
            nc.scalar.dma_start(blob_e[:, :], blob_e_d[:, :])

            w1_sb = blob_a[0:3, OFF_W1:OFF_W1 + 256]
            mlp_sb = blob_a[0:3, OFF_MLP:OFF_MLP + 16]
            bias = blob_a.bitcast(F32)[:, 0:2]

            def chunk_region(c):
                if c == 0:
                    return blob_a, COLS_W1X
                if c <= 2:
                    return blob_b, (c - 1) * CHUNK_COLS[1]
                return blob_e, sum(CHUNK_COLS[3:3 + (c - 3)])

            # ---- PE warm-up / fillers ----
            rgb_ps = ps_rgb.tile([48, NPOS], F32, tag="rgb")
            warm_ps = ps_warm.tile([128, 128], F32, tag="warm_ps")
            warm = work.tile([128, 512], BF16, tag="warm")
            nc.gpsimd.memset(warm[:, :], 0.0)

            def filler(n, dep=None):
                for _ in range(n):
                    if dep is None:
                        nc.tensor.matmul(
                            warm_ps[:, :], warm[:, 0:128], warm[:, 0:128],
                            start=True, stop=True,
                        )
                    else:
                        k, mcols = dep.shape
                        nc.tensor.matmul(
                            warm_ps[0:mcols, 0:128], dep, warm[0:k, 0:128],
                            start=True, stop=True,
                        )

            for _ in range(warm_big):
                nc.tensor.matmul(
                    rgb_ps[:, :], warm[:, 0:48], warm[:, 0:NPOS],
                    start=True, stop=True,
                )
            filler(warm_small)

            # ---- MLP layer 1: h_actT [256, 16] in two 128-chunks ----
            h_sb = work.tile([128, 32], BF16, tag="hact")
            for hc in range(2):
                ph = ps_small.tile([128, 16], F32, tag="ph")
                nc.tensor.matmul(
                    ph[:, :], w1_sb[:, hc * 128:(hc + 1) * 128], mlp_sb[:, :],
                    start=True, stop=True,
                )
                # relu(x + b1) = max(x + b1, 0) in one DVE op
                nc.vector.tensor_scalar(
                    h_sb[:, hc * 16:(hc + 1) * 16], ph[:, :],
                    bias[:, hc:hc + 1], 0.0,
                    mybir.AluOpType.add, mybir.AluOpType.max,
                )
            filler(fills[0], dep=h_sb[:, 0:32])

            # ---- W assembly (h-contraction on PE; bias + bf16 cast in ONE
            # DVE tensor_add) with conv lagging one chunk behind ----
            def asm_chunk(c):
                taps, plo, roff, K = CHUNK_SPECS[c]
                blob, base = chunk_region(c)
                pw = ps_w.tile([128, 48], F32, tag="pw")
                for o in range(3):
                    for hc in range(2):
                        nc.tensor.matmul(
                            pw[:K, o * 16:(o + 1) * 16],
                            blob[:, base + (o * 2 + hc) * K:base + (o * 2 + hc + 1) * K],
                            h_sb[:, hc * 16:(hc + 1) * 16],
                            start=(hc == 0), stop=(hc == 1),
                        )
                b2_tile = blob[0:K, base + 6 * K:base + 6 * K + 48]
                w_sb = wpool.tile([128, 48], BF16, tag="W")
                nc.vector.tensor_add(w_sb[:K, :], pw[:K, :], b2_tile)
                return w_sb

            def conv_chunk(c, w_sb, start, stop):
                taps, plo, roff, K = CHUNK_SPECS[c]
                rhs = blob_band[plo:plo + K, roff:roff + 8 * 66].rearrange(
                    "p (r c) -> p r c", c=66
                )[:, :, 0:64]
                nc.tensor.matmul(
                    rgb_ps[:, :], w_sb[:K, :], rhs, start=start, stop=stop,
                )

            # Chunk processing follows blob ARRIVAL order -- c0 (Sync blob A),
            # c3..c5 (Scalar blob E), then c1/c2 (Sync blob B, last to land)
            # -- so the post-stream endgame is as short as possible.
            SEQ = [0, 3, 4, 5, 1, 2]
            w_tiles = {}
            w_tiles[SEQ[0]] = asm_chunk(SEQ[0])
            filler(fills[1], dep=h_sb[:, 0:32])
            for i in range(1, N_CHUNKS):
                w_tiles[SEQ[i]] = asm_chunk(SEQ[i])
                filler(fills[1 + i], dep=h_sb[:, 0:32])
                conv_chunk(SEQ[i - 1], w_tiles[SEQ[i - 1]],
                           start=(i == 1), stop=False)
            conv_chunk(SEQ[-1], w_tiles[SEQ[-1]], start=False, stop=True)

            # ---- write out (bf16, host upcasts) in two overlapped
            # copy+DMA halves on the warm Sync queue ----
            out_sb = opool.tile([48, NPOS], BF16, tag="out")
            half = NPOS // 2
            nc.vector.tensor_copy(out_sb[:, 0:half], rgb_ps[:, 0:half])
            nc.sync.dma_start(out48[:, 0:half], out_sb[:, 0:half])
            nc.vector.tensor_copy(out_sb[:, half:], rgb_ps[:, half:])
            nc.sync.dma_start(out48[:, half:], out_sb[:, half:])

    nc.compile()
    return nc


def _host_prep(feat, w1, b1, w2, b2):
    """Pack shared blobs + per-core band blobs (bf16)."""
    import ml_dtypes
    bf16 = ml_dtypes.bfloat16
    feat = np.ascontiguousarray(np.asarray(feat, dtype=np.float32))[0]  # [64,64,64]
    w1 = np.asarray(w1, dtype=np.float32)
    b1 = np.asarray(b1, dtype=np.float32)
    w2 = np.asarray(w2, dtype=np.float32)
    b2 = np.asarray(b2, dtype=np.float32)

    dydx = np.arange(16)
    mlpin = np.stack(
        [dydx // 4 / 4.0, dydx % 4 / 4.0, np.full(16, 0.25)], axis=0
    ).astype(np.float32)  # [3, 16]

    w2r = w2.reshape(256, 64, 9, 3).astype(bf16)  # [h, c, t, o]
    b2r = b2.reshape(64, 9, 3)                    # [c, t, o]

    bias = np.zeros((128, 2), dtype=np.float32)
    bias[:, 0] = b1[0:128]
    bias[:, 1] = b1[128:256]

    def chunk_block(c):
        """[128, 6K+48] w2 + b2 tile for chunk c."""
        taps, plo, roff, K = CHUNK_SPECS[c]
        blk = np.zeros((128, 6 * K + 48), dtype=bf16)
        for o in range(3):
            for hc in range(2):
                block = np.concatenate(
                    [w2r[hc * 128:(hc + 1) * 128, :, t, o] for t in taps], axis=1
                )
                blk[:, (o * 2 + hc) * K:(o * 2 + hc + 1) * K] = block
            col = np.concatenate([b2r[:, t, o] for t in taps])  # [K]
            blk[0:K, 6 * K + o * 16:6 * K + (o + 1) * 16] = \
                col[:, None].astype(bf16)
        return blk

    blob_a = np.zeros((128, COLS_A), dtype=bf16)
    blob_a[:, 0:4] = bias.view(bf16)
    blob_a[0:3, OFF_W1:OFF_W1 + 256] = w1.astype(bf16)
    blob_a[0:3, OFF_MLP:OFF_MLP + 16] = mlpin.astype(bf16)
    blob_a[:, COLS_W1X:] = chunk_block(0)

    blob_b = np.concatenate([chunk_block(1), chunk_block(2)], axis=1)
    blob_e = np.concatenate([chunk_block(3), chunk_block(4), chunk_block(5)], axis=1)

    featp = np.zeros((64, 66, 66), dtype=bf16)
    featp[:, 1:65, 1:65] = feat.astype(bf16)

    blobs_band = []
    for core in range(N_CORES):
        r0 = core * ROWS_PER_CORE
        band = featp[:, r0:r0 + BAND_ROWS, :].reshape(64, BAND_ROWS * 66)
        bb = np.zeros((128, COLS_BAND), dtype=bf16)
        bb[0:64, 1:661] = band
        bb[64:128, 0:660] = band
        blobs_band.append(bb)
    return blob_a, blob_b, blob_e, blobs_band


def _assemble(per_core_out48):
    """[8 x [48, 512] bf16] -> [1, 3, 256, 256] f32."""
    full = np.stack([np.asarray(o).astype(np.float32) for o in per_core_out48])
    full = full.reshape(8, 3, 4, 4, 8, 64)               # [core, o, dy, dx, r, x]
    rgb = full.transpose(1, 0, 4, 2, 5, 3).reshape(3, 256, 256)
    return np.ascontiguousarray(rgb)[None]


def get_program():
    cfg = (WARM_BIG, WARM_SMALL, tuple(FILLS))
    if cfg not in _CACHE:
        _CACHE[cfg] = _build_program(cfg)
    return _CACHE[cfg]


def run(feat, w1, b1, w2, b2, out_h, out_w, trace=False, **spmd_kwargs):
    assert int(out_h) == 256 and int(out_w) == 256
    nc = get_program()
    blob_a, blob_b, blob_e, blobs_band = _host_prep(feat, w1, b1, w2, b2)
    in_maps = [
        {"blob_a": blob_a, "blob_b": blob_b, "blob_e": blob_e,
         "blob_band": blobs_band[core]}
        for core in range(N_CORES)
    ]
    res = run_bass_kernel_spmd(
        nc, in_maps, core_ids=list(range(N_CORES)), trace=trace, **spmd_kwargs
    )
    out = _assemble([res.results[core]["out48"] for core in range(N_CORES)])
    return out, res


def kernel(feat, w1, b1, w2, b2, out_h, out_w):
    out, _ = run(feat, w1, b1, w2, b2, out_h, out_w, trace=False)
    return out


# revision 26
# speedup vs baseline: 1.1150x; 1.1150x over previous
"""MetaSR super-resolution Trainium2 kernel (bf16, dual-queue, hybrid taps).

Structure exploited: out_h=out_w=256 with H=W=64 LR grid means the scale
factor is exactly 4, so the nearest-neighbor gather index is iy=oy//4,
ix=ox//4 and the per-query MLP input collapses to 16 distinct subpixel
phases [dy/4, dx/4, 0.25].  The whole model becomes:

  1. h    = relu(mlp_in @ w1 + b1)              [16, 256]
  2. predw = h @ w2 + b2                        [16, 576, 3]
  3. rgb[o, 4*iy+dy, 4*ix+dx] =
       sum_{c,ki,kj} feat[c, iy+ki-1, ix+kj-1] * predw[(dy,dx), c*9+ki*3+kj, o]
     i.e. a 3x3 conv with 64 in / 48 out channels + pixel shuffle.

Sharding: data-parallel over LR rows (8 rows per core, 10-row halo band),
weights replicated; steps 1+2 are recomputed on every core (tiny).

Conv decomposition: one [128, 663] band tile holds the zero-padded band
twice -- partitions 0-63 at free-offset 1, partitions 64-127 at offset 0.
That single tile supports K=128 tap pairs with shift delta 1 ([t0;t1],
[t4;t5], [t6;t7]) and the remaining taps t2/t3/t8 singly (K=64 from the
offset-1 half), for 6 conv matmuls off one 169KB band transfer.

The b2 bias rides each w2 chunk blob as a prebroadcast [K, 48] tile; one
DVE tensor_add per chunk applies bias + bf16 cast, and each chunk's conv
is issued one chunk BEHIND its assembly (software pipelining) so that DVE
op is never on the PE's critical path.

All large operands are bf16 (host-side cast); verified end-to-end rel err
~4e-3 vs the f32 reference (budget 2e-2).

DMA: measured ~215-230GB/s per queue with ~0.3us per-transfer bubbles, and
near-360GB/s aggregate when BOTH hardware queues stream concurrently.  So:
Sync carries [w1+mlp+b1 | w2 chunk0] then [chunk1|chunk2] then the output;
Scalar carries [band] then [chunks 3-5], but its first trigger is GATED on
the arrival of the first Sync blob (a 1-element scalar-engine copy) so the
critical MLP weights always win the engine race.  Dummy "filler" matmuls
with explicit deps keep the PE busy across the remaining DMA waits.
"""

import os

import numpy as np

try:
    import concourse.bass as bass
except ImportError:  # fall back to the repo checkout
    import sys
    sys.path.insert(0, "/opt/trn_rl_repo")
    import concourse.bass as bass
import concourse.mybir as mybir
import concourse.tile as tile
from concourse import bacc
from concourse.bass_utils import run_bass_kernel_spmd

F32 = mybir.dt.float32
BF16 = mybir.dt.bfloat16
N_CORES = 8
ROWS_PER_CORE = 8          # LR rows per core
BAND_ROWS = ROWS_PER_CORE + 2
NPOS = ROWS_PER_CORE * 64  # 512 LR positions per core

# Taps t = ki*3+kj have band shift ki*66+kj:
#   t:      0   1   2   3    4    5    6    7    8
#   shift:  0   1   2   66   67   68   132  133  134
# chunks: ([taps], part_lo, rhs_offset, K)
CHUNK_SPECS = [
    ([0, 1], 0, 1, 128),
    ([4, 5], 0, 68, 128),
    ([6, 7], 0, 133, 128),
    ([2], 0, 3, 64),
    ([3], 0, 67, 64),
    ([8], 0, 135, 64),
]
N_CHUNKS = len(CHUNK_SPECS)

# blob layout (bf16 cols); per-chunk w2 block = 6*[128,K] + [K,48] b2 tile
OFF_W1 = 4            # b1 takes 2 f32 = 4 bf16 cols
OFF_MLP = 4 + 256
COLS_W1X = 4 + 256 + 16           # = 276
CHUNK_COLS = [6 * spec[3] + 48 for spec in CHUNK_SPECS]   # 816/816/816/432/432/432
COLS_A = COLS_W1X + CHUNK_COLS[0]          # w1x | chunk0
COLS_B = CHUNK_COLS[1] + CHUNK_COLS[2]     # chunk1 | chunk2
COLS_E = CHUNK_COLS[3] + CHUNK_COLS[4] + CHUNK_COLS[5]
COLS_BAND = 663


def _env(name, default):
    return int(os.environ.get(name, str(default)))

WARM_BIG = _env("METASR_WARM_BIG", 2)
WARM_SMALL = _env("METASR_WARM_SMALL", 2)
# filler counts: after mlp1, then after each chunk's W assembly
FILLS = [int(x) for x in os.environ.get("METASR_FILLS", "3,2,1,1,1,1,0").split(",")]

_CACHE = {}


def _build_program(cfg):
    """Build + compile the single-core Bass program (same for all cores)."""
    warm_big, warm_small, fills = cfg[0], cfg[1], list(cfg[2])
    nc = bacc.Bacc("TRN2", target_bir_lowering=False, debug=False)

    blob_a_d = nc.dram_tensor("blob_a", [128, COLS_A], BF16, kind="ExternalInput")
    blob_b_d = nc.dram_tensor("blob_b", [128, COLS_B], BF16, kind="ExternalInput")
    blob_band_d = nc.dram_tensor("blob_band", [128, COLS_BAND], BF16, kind="ExternalInput")
    blob_e_d = nc.dram_tensor("blob_e", [128, COLS_E], BF16, kind="ExternalInput")
    out48 = nc.dram_tensor("out48", [48, NPOS], BF16, kind="ExternalOutput")

    with tile.TileContext(nc) as tc:
        with (
            tc.tile_pool(name="blobs", bufs=1) as blobs,
            tc.tile_pool(name="work", bufs=1) as work,
            tc.tile_pool(name="wpool", bufs=6) as wpool,
            tc.tile_pool(name="opool", bufs=1) as opool,
            tc.tile_pool(name="ps_small", bufs=2, space="PSUM") as ps_small,
            tc.tile_pool(name="ps_w", bufs=4, space="PSUM") as ps_w,
            tc.tile_pool(name="ps_warm", bufs=1, space="PSUM") as ps_warm,
            tc.tile_pool(name="ps_rgb", bufs=1, space="PSUM") as ps_rgb,
        ):
            # Sync queue: critical stream.
            blob_a = blobs.tile([128, COLS_A], BF16, tag="blob_a")
            nc.sync.dma_start(blob_a[:, :], blob_a_d[:, :])
            blob_b = blobs.tile([128, COLS_B], BF16, tag="blob_b")
            nc.sync.dma_start(blob_b[:, :], blob_b_d[:, :])
            # Scalar queue: gated on blob_a's arrival so Sync wins the race.
            # The gate writes INTO the band tile, giving the band DMA a WAW
            # dependency the scheduler cannot hoist the trigger above.  The
            # written cell [127, 661] is outside every conv rhs slice.
            blob_band = blobs.tile([128, COLS_BAND], BF16, tag="band")
            nc.vector.tensor_copy(blob_band[0:1, 661:662], blob_a[0:1, 0:1])
            nc.scalar.dma_start(blob_band[:, :], blob_band_d[:, :])
            blob_e = blobs.tile([128, COLS_E], BF16, tag="blob_e")
            nc.scalar.dma_start(blob_e[:, :], blob_e_d[:, :])

            w1_sb = blob_a[0:3, OFF_W1:OFF_W1 + 256]
            mlp_sb = blob_a[0:3, OFF_MLP:OFF_MLP + 16]
            bias = blob_a.bitcast(F32)[:, 0:2]

            def chunk_region(c):
                if c == 0:
                    return blob_a, COLS_W1X
                if c <= 2:
                    return blob_b, (c - 1) * CHUNK_COLS[1]
                return blob_e, sum(CHUNK_COLS[3:3 + (c - 3)])

            # ---- PE warm-up / fillers ----
            rgb_ps = ps_rgb.tile([48, NPOS], F32, tag="rgb")
            warm_ps = ps_warm.tile([128, 128], F32, tag="warm_ps")
            warm = work.tile([128, 512], BF16, tag="warm")
            nc.gpsimd.memset(warm[:, :], 0.0)

            def filler(n, dep=None):
                for _ in range(n):
                    if dep is None:
                        nc.tensor.matmul(
                            warm_ps[:, :], warm[:, 0:128], warm[:, 0:128],
                            start=True, stop=True,
                        )
                    else:
                        k, mcols = dep.shape
                        nc.tensor.matmul(
                            warm_ps[0:mcols, 0:128], dep, warm[0:k, 0:128],
                            start=True, stop=True,
                        )

            for _ in range(warm_big):
                nc.tensor.matmul(
                    rgb_ps[:, :], warm[:, 0:48], warm[:, 0:NPOS],
                    start=True, stop=True,
                )
            filler(warm_small)

            # ---- MLP layer 1: h_actT [256, 16] in two 128-chunks ----
            h_sb = work.tile([128, 32], BF16, tag="hact")
            for hc in range(2):
                ph = ps_small.tile([128, 16], F32, tag="ph")
                nc.tensor.matmul(
                    ph[:, :], w1_sb[:, hc * 128:(hc + 1) * 128], mlp_sb[:, :],
                    start=True, stop=True,
                )
                # relu(x + b1) = max(x + b1, 0) in one DVE op
                nc.vector.tensor_scalar(
                    h_sb[:, hc * 16:(hc + 1) * 16], ph[:, :],
                    bias[:, hc:hc + 1], 0.0,
                    mybir.AluOpType.add, mybir.AluOpType.max,
                )
            filler(fills[0], dep=h_sb[:, 0:32])

            # ---- W assembly (h-contraction on PE; bias + bf16 cast in ONE
            # DVE tensor_add) with conv lagging one chunk behind ----
            def asm_chunk(c):
                taps, plo, roff, K = CHUNK_SPECS[c]
                blob, base = chunk_region(c)
                pw = ps_w.tile([128, 48], F32, tag="pw")
                for o in range(3):
                    for hc in range(2):
                        nc.tensor.matmul(
                            pw[:K, o * 16:(o + 1) * 16],
                            blob[:, base + (o * 2 + hc) * K:base + (o * 2 + hc + 1) * K],
                            h_sb[:, hc * 16:(hc + 1) * 16],
                            start=(hc == 0), stop=(hc == 1),
                        )
                b2_tile = blob[0:K, base + 6 * K:base + 6 * K + 48]
                w_sb = wpool.tile([128, 48], BF16, tag="W")
                nc.vector.tensor_add(w_sb[:K, :], pw[:K, :], b2_tile)
                return w_sb

            def conv_chunk(c, w_sb, start, stop):
                taps, plo, roff, K = CHUNK_SPECS[c]
                rhs = blob_band[plo:plo + K, roff:roff + 8 * 66].rearrange(
                    "p (r c) -> p r c", c=66
                )[:, :, 0:64]
                nc.tensor.matmul(
                    rgb_ps[:, :], w_sb[:K, :], rhs, start=start, stop=stop,
                )

            # Chunk processing follows blob ARRIVAL order (A then B on Sync,
            # E last on the gated Scalar queue) so the post-stream endgame is
            # as short as possible.
            SEQ = [0, 1, 2, 3, 4, 5]
            w_tiles = {}
            w_tiles[SEQ[0]] = asm_chunk(SEQ[0])
            filler(fills[1], dep=h_sb[:, 0:32])
            for i in range(1, N_CHUNKS):
                w_tiles[SEQ[i]] = asm_chunk(SEQ[i])
                filler(fills[1 + i], dep=h_sb[:, 0:32])
                conv_chunk(SEQ[i - 1], w_tiles[SEQ[i - 1]],
                           start=(i == 1), stop=False)
            conv_chunk(SEQ[-1], w_tiles[SEQ[-1]], start=False, stop=True)

            # ---- write out (bf16, host upcasts) on the warm Sync queue ----
            out_sb = opool.tile([48, NPOS], BF16, tag="out")
            nc.vector.tensor_copy(out_sb[:, :], rgb_ps[:, :])
            nc.sync.dma_start(out48[:, :], out_sb[:, :])

    nc.compile()
    return nc


def _host_prep(feat, w1, b1, w2, b2):
    """Pack shared blobs + per-core band blobs (bf16)."""
    import ml_dtypes
    bf16 = ml_dtypes.bfloat16
    feat = np.ascontiguousarray(np.asarray(feat, dtype=np.float32))[0]  # [64,64,64]
    w1 = np.asarray(w1, dtype=np.float32)
    b1 = np.asarray(b1, dtype=np.float32)
    w2 = np.asarray(w2, dtype=np.float32)
    b2 = np.asarray(b2, dtype=np.float32)

    dydx = np.arange(16)
    mlpin = np.stack(
        [dydx // 4 / 4.0, dydx % 4 / 4.0, np.full(16, 0.25)], axis=0
    ).astype(np.float32)  # [3, 16]

    w2r = w2.reshape(256, 64, 9, 3).astype(bf16)  # [h, c, t, o]
    b2r = b2.reshape(64, 9, 3)                    # [c, t, o]

    bias = np.zeros((128, 2), dtype=np.float32)
    bias[:, 0] = b1[0:128]
    bias[:, 1] = b1[128:256]

    def chunk_block(c):
        """[128, 6K+48] w2 + b2 tile for chunk c."""
        taps, plo, roff, K = CHUNK_SPECS[c]
        blk = np.zeros((128, 6 * K + 48), dtype=bf16)
        for o in range(3):
            for hc in range(2):
                block = np.concatenate(
                    [w2r[hc * 128:(hc + 1) * 128, :, t, o] for t in taps], axis=1
                )
                blk[:, (o * 2 + hc) * K:(o * 2 + hc + 1) * K] = block
            col = np.concatenate([b2r[:, t, o] for t in taps])  # [K]
            blk[0:K, 6 * K + o * 16:6 * K + (o + 1) * 16] = \
                col[:, None].astype(bf16)
        return blk

    blob_a = np.zeros((128, COLS_A), dtype=bf16)
    blob_a[:, 0:4] = bias.view(bf16)
    blob_a[0:3, OFF_W1:OFF_W1 + 256] = w1.astype(bf16)
    blob_a[0:3, OFF_MLP:OFF_MLP + 16] = mlpin.astype(bf16)
    blob_a[:, COLS_W1X:] = chunk_block(0)

    blob_b = np.concatenate([chunk_block(1), chunk_block(2)], axis=1)
    blob_e = np.concatenate([chunk_block(3), chunk_block(4), chunk_block(5)], axis=1)

    featp = np.zeros((64, 66, 66), dtype=bf16)
    featp[:, 1:65, 1:65] = feat.astype(bf16)

    blobs_band = []
    for core in range(N_CORES):
        r0 = core * ROWS_PER_CORE
        band = featp[:, r0:r0 + BAND_ROWS, :].reshape(64, BAND_ROWS * 66)
        bb = np.zeros((128, COLS_BAND), dtype=bf16)
        bb[0:64, 1:661] = band
        bb[64:128, 0:660] = band
        blobs_band.append(bb)
    return blob_a, blob_b, blob_e, blobs_band


def _assemble(per_core_out48):
    """[8 x [48, 512] bf16] -> [1, 3, 256, 256] f32."""
    full = np.stack([np.asarray(o).astype(np.float32) for o in per_core_out48])
    full = full.reshape(8, 3, 4, 4, 8, 64)               # [core, o, dy, dx, r, x]
    rgb = full.transpose(1, 0, 4, 2, 5, 3).reshape(3, 256, 256)
    return np.ascontiguousarray(rgb)[None]


def get_program():
    cfg = (WARM_BIG, WARM_SMALL, tuple(FILLS))
    if cfg not in _CACHE:
        _CACHE[cfg] = _build_program(cfg)
    return _CACHE[cfg]


def run(feat, w1, b1, w2, b2, out_h, out_w, trace=False, **spmd_kwargs):
    assert int(out_h) == 256 and int(out_w) == 256
    nc = get_program()
    blob_a, blob_b, blob_e, blobs_band = _host_prep(feat, w1, b1, w2, b2)
    in_maps = [
        {"blob_a": blob_a, "blob_b": blob_b, "blob_e": blob_e,
         "blob_band": blobs_band[core]}
        for core in range(N_CORES)
    ]
    res = run_bass_kernel_spmd(
        nc, in_maps, core_ids=list(range(N_CORES)), trace=trace, **spmd_kwargs
    )
    out = _assemble([res.results[core]["out48"] for core in range(N_CORES)])
    return out, res


def kernel(feat, w1, b1, w2, b2, out_h, out_w):
    out, _ = run(feat, w1, b1, w2, b2, out_h, out_w, trace=False)
    return out
